# revision 19
# baseline (speedup 1.0000x reference)
"""Trainium2 Bass kernel for nn_CrossFusionModule (sparse_attention).

Computation (per batch b, query q):
  q2 = q @ wq.T + bq ; k2 = wk*k + bk  (1x1 conv)
  w[q,n,s] = (q2/sqrt(dh))[q,n,:] . k2[n,:,s] ; softmax over (n,s); ws = sum_n
  xf = x * ws[q]   (per-pixel scale of the [512,1024] feature map)
  RefineModule: x0 = GN(conv1x1_b0(xf)); x1 = chain of convs+GN;
  xc = GN(conv3x3_cat([x0,x1])); xr = GN(conv1x1_res(xf)); out = relu(xc+xr)

Sharding: 8 cores; core i handles b = i//4 and 25 queries (i%4)*25..+25.
Key trick: conv1x1(x*ws) = conv1x1(x)*ws, so Y_j = conv1x1_j(x) is computed
once per core and only scaled per query; their GN stats are computed with
tiny matmuls against ws and ws^2 (group colsums hoisted per batch).
"""

import sys
import types

import numpy as np

for p in ("/opt/trn_rl_repo", "/root/.axon_site"):
    if p not in sys.path:
        sys.path.insert(0, p)

import concourse.bass as bass
import concourse.tile as tile
from concourse import mybir
from concourse.bass_utils import run_bass_kernel_spmd
from concourse.vector_clock import ScopedClock

F32 = mybir.dt.float32
AF = mybir.ActivationFunctionType
OP = mybir.AluOpType

B, Q, QD, HD, NH, HW = 2, 100, 256, 256, 8, 32
S = HW * HW  # 1024
CH = 64
OC = 2 * CH  # 128
INC = 512
GN_GROUPS = 8
EPS = 1e-5
NCORES = 8
NQ = (B * Q) // NCORES  # 25 queries per core
GSZ = OC // GN_GROUPS  # 16 channels per group
NG_INV = 1.0 / (GSZ * S)  # 1/16384


def _patch_tile_drain():
    """Work around walrus 'Too many sync wait commands' on the kernel-tail
    Drain: spill its semaphore waits onto individual SP nops."""

    def _drain_and_barrier(self, tick_clock, wait_clock):
        drain_inst = self.nc.sync.drain()
        wait_clock.add_sem_waits(
            drain_inst.ins, ScopedClock({None: tick_clock.global_clock})
        )
        mi = drain_inst.ins
        waits = list(mi.sync_info.on_wait) if mi.sync_info is not None else []
        if len(waits) > 1:
            del mi.sync_info.on_wait[:]
            bb = self.nc.cur_bb.bb
            bb.instructions.pop()
            for w in waits:
                nop = self.nc.sync.nop(nofuse=True, hint="drain_wait_spill")
                if nop.ins.sync_info is None:
                    nop.ins.sync_info = mybir.SyncInfo(on_wait=[w], on_update=[])
                else:
                    nop.ins.sync_info.on_wait.append(w)
            bb.instructions.append(mi)
        self.nc.all_engine_barrier()
        popped = self.nc._tile_sem_poison_stack.pop()
        assert popped is self._sem_poison
        self.nc.clear_and_free_semaphores(list(self.sems.allocated().values()))
        self.nc.all_engine_barrier()

    tile.TileContext._drain_and_barrier = _drain_and_barrier


def _register_ntff_hook():
    """The trimmed container lacks antenv.axon_hooks; register the NTFF
    profiling hook manually so trace=True works (used by test.py only)."""
    if "antenv.axon_hooks" in sys.modules:
        return
    try:
        from trn_agent_boot.trn_boot import _ntff_profile_via_ctypes

        hook = _ntff_profile_via_ctypes("/opt/axon/libaxon_pjrt.so")
    except Exception:
        hook = None
    mod = types.ModuleType("antenv.axon_hooks")
    mod.get_axon_ntff_profile_hook = lambda: hook
    mod.set_axon_ntff_profile_hook = lambda h: None
    sys.modules["antenv.axon_hooks"] = mod


_patch_tile_drain()
_register_ntff_hook()

_DMA_INSTS = tuple(
    getattr(mybir, n)
    for n in ("InstDMACopy", "InstDmaTransposeAnt", "InstDMAGatherAnt",
              "InstDMAScatterAddAnt", "InstCollectiveCompute")
    if hasattr(mybir, n)
)


def _spill_waits(nc, maxw=1):
    """This walrus build supports only one semaphore wait per engine
    instruction and does not auto-split; move excess waits onto nops."""
    nid = 0
    for f in nc.m.functions:
        for blk in f.blocks:
            new = []
            for inst in blk.instructions:
                si = getattr(inst, "sync_info", None)
                if si is not None and si.on_wait and len(si.on_wait) > maxw:
                    waits = list(si.on_wait)
                    del si.on_wait[:]
                    si.on_wait.extend(waits[-maxw:])
                    for w in waits[:-maxw]:
                        nop = mybir.InstNoOp(
                            name=f"I-wspill-{nid}",
                            engine=inst.engine,
                            sync_info=mybir.SyncInfo(on_wait=[w], on_update=[]),
                            bass_nofuse=True,
                        )
                        nid += 1
                        nc.register_instruction(nop)
                        new.append(nop)
                new.append(inst)
            blk.instructions[:] = new


# ----------------------------------------------------------------------------
# host-side weight prep
# ----------------------------------------------------------------------------

def _prep_consts(params):
    """Transform weights into the SBUF layouts the kernel consumes."""
    nf = float(HD / NH) ** (-0.5)
    c = {}

    def npf(a):
        return np.ascontiguousarray(np.asarray(a, dtype=np.float32))

    # wq/wk: [HD, QD] -> [kc, p, mo, o]: w[mo*128+o, kc*128+p]
    for name, w, bvec, scale in (
        ("wq", params["wq"], params["bq"], nf),
        ("wk", params["wk"], params["bk"], 1.0),
    ):
        wt = npf(w) * scale  # [256 o, 256 c]
        # [kc, p, mo, o] -> [p, kc, mo, o]
        c[name + "T"] = npf(wt.T.reshape(2, 128, 2, 128).transpose(1, 0, 2, 3))
        bv = npf(np.asarray(bvec, np.float32) * scale).reshape(2, 128)
        c["b" + name[1]] = npf(bv.T[:, :, None])  # [o(p), mo, 1]

    # conv weights: w [O, I, kh, kw] -> WT [p, kc, tap, o] with c = kc*128+p
    def convT(p):
        w = npf(p["w"])  # [O, I, kh, kw]
        O_, I_, kh, kw = w.shape
        kc = I_ // 128
        # [p, kc, tap, o]
        wt = w.reshape(O_, kc, 128, kh * kw).transpose(2, 1, 3, 0)
        return npf(wt)

    c["w0T"] = convT(params["b0"])      # [128, 4, 1, 128]
    c["w1c0T"] = convT(params["b1c0"])  # [128, 4, 1, 128]
    c["wresT"] = convT(params["res"])   # [128, 4, 1, 128]
    c["w1c1T"] = convT(params["b1c1"])  # [128, 1, 3, 128]
    c["w1c2T"] = convT(params["b1c2"])  # [128, 1, 3, 128]
    c["w1c3T"] = convT(params["b1c3"])  # [128, 1, 9, 128]
    c["wcatT"] = convT(params["cat"])   # [128, 2, 9, 128]

    # GN gamma/beta columns: [128, 7]; order: b0,b1c0,b1c1,b1c2,b1c3,cat,res
    order = ["b0", "b1c0", "b1c1", "b1c2", "b1c3", "cat", "res"]
    c["gn_g"] = npf(np.stack([npf(params[k]["g"]) for k in order], axis=1))
    c["gn_b"] = npf(np.stack([npf(params[k]["b"]) for k in order], axis=1))

    # constants
    c["ident"] = npf(np.eye(128, dtype=np.float32))
    c["ones"] = npf(np.ones((128, 128), np.float32))
    g8 = np.zeros((128, 8), np.float32)
    for i in range(128):
        g8[i, i // GSZ] = NG_INV
    c["g8s"] = npf(g8)  # scaled group indicator (for colsums -> means)
    g8bc = np.zeros((8, 128), np.float32)
    for i in range(128):
        g8bc[i // GSZ, i] = 1.0
    c["g8bc"] = npf(g8bc)
    c["g8bcn"] = npf(-g8bc)
    g128 = np.zeros((128, 128), np.float32)
    for i in range(128):
        g0 = (i // GSZ) * GSZ
        g128[g0 : g0 + GSZ, i] = 1.0 / GSZ
    # [c, p]: out[p] = mean over group(p) of per-partition stats (1/16 each)
    c["g128s"] = npf(g128)
    gq = np.zeros((128, NQ), np.float32)
    for j in range(4):
        for i in range(NQ):
            gq[32 * j + i, i] = 1.0
    c["gq"] = npf(gq)
    c["eps128"] = npf(np.full((128, 1), EPS, np.float32))
    c["eps8"] = npf(np.full((8, 1), EPS, np.float32))
    return c


CONST_NAMES = [
    "wqT", "bq", "wkT", "bk",
    "w0T", "w1c0T", "wresT", "w1c1T", "w1c2T", "w1c3T", "wcatT",
    "gn_g", "gn_b",
    "ident", "ones", "g8s", "g8bc", "g8bcn", "g128s", "gq", "eps128", "eps8",
]


# ----------------------------------------------------------------------------
# device program
# ----------------------------------------------------------------------------

def build_program(nq=NQ):
    nc = bass.Bass("TRN2", target_bir_lowering=False)

    # ---- DRAM I/O ----
    d = {}
    d["xk"] = nc.dram_tensor("xk", [INC, S], F32, kind="ExternalInput")
    d["kk"] = nc.dram_tensor("kk", [QD, S], F32, kind="ExternalInput")
    d["qT"] = nc.dram_tensor("qT", [QD, nq], F32, kind="ExternalInput")
    d["mrow"] = nc.dram_tensor("mrow", [1, S], F32, kind="ExternalInput")
    shapes = {
        "wqT": [128, 2, 2, 128], "bq": [128, 2, 1],
        "wkT": [128, 2, 2, 128], "bk": [128, 2, 1],
        "w0T": [128, 4, 1, 128], "w1c0T": [128, 4, 1, 128],
        "wresT": [128, 4, 1, 128],
        "w1c1T": [128, 1, 3, 128], "w1c2T": [128, 1, 3, 128],
        "w1c3T": [128, 1, 9, 128], "wcatT": [128, 2, 9, 128],
        "gn_g": [128, 7], "gn_b": [128, 7],
        "ident": [128, 128], "ones": [128, 128],
        "g8s": [128, 8], "g8bc": [8, 128], "g8bcn": [8, 128],
        "g128s": [128, 128], "gq": [128, nq],
        "eps128": [128, 1], "eps8": [8, 1],
    }
    for n in CONST_NAMES:
        d[n] = nc.dram_tensor(n, shapes[n], F32, kind="ExternalInput")
    out_h = nc.dram_tensor("out", [nq, OC, S], F32, kind="ExternalOutput")

    with tile.TileContext(nc) as tc:
        _emit(nc, tc, d, out_h, nq)
    _spill_waits(nc)
    return nc


def _emit(nc, tc, d, out_h, nq):
    import contextlib

    ctx = contextlib.ExitStack()
    cp = ctx.enter_context(tc.tile_pool(name="const", bufs=1))
    qp = ctx.enter_context(tc.tile_pool(name="work", bufs=2))
    tp = ctx.enter_context(tc.tile_pool(name="tiny", bufs=4))
    pp = ctx.enter_context(tc.tile_pool(name="psum", bufs=1, space="PSUM"))

    # ---- load constants ----
    sb = {}
    for n in ("wqT", "bq", "wkT", "bk", "w0T", "w1c0T", "wresT", "w1c1T",
              "w1c2T", "w1c3T", "wcatT", "gn_g", "gn_b", "ident", "ones",
              "g8s", "g8bc", "g8bcn", "g128s", "gq", "eps128", "eps8"):
        t = cp.tile(list(d[n].shape), F32, name="sb_" + n)
        nc.sync.dma_start(out=t, in_=d[n][...])
        sb[n] = t

    # inputs
    x_sb = cp.tile([128, 4, S], F32, name="x_sb")
    nc.sync.dma_start(out=x_sb, in_=d["xk"][...].rearrange("(kc p) s -> p kc s", p=128))
    k_sb = cp.tile([128, 2, S], F32, name="k_sb")
    nc.sync.dma_start(out=k_sb, in_=d["kk"][...].rearrange("(kc p) s -> p kc s", p=128))
    qT_sb = cp.tile([128, 2, nq], F32, name="qT_sb")
    nc.sync.dma_start(out=qT_sb, in_=d["qT"][...].rearrange("(kc p) q -> p kc q", p=128))
    msk_sb = cp.tile([1, S], F32, name="msk_sb")
    nc.sync.dma_start(out=msk_sb, in_=d["mrow"][...])

    MM = nc.tensor.matmul

    # ================= attention preamble =================
    # k2 = wk @ k + bk -> kh_sb [128, 2(mo), 1024]
    kh_sb = cp.tile([128, 2, S], F32, name="kh_sb")
    for mo in range(2):
        ps = pp.tile([128, S], F32, tag="big", bufs=3)
        for kc in range(2):
            for ncol in range(2):
                MM(ps[:, ncol * 512 : (ncol + 1) * 512],
                   sb["wkT"][:, kc, mo, :],
                   k_sb[:, kc, ncol * 512 : (ncol + 1) * 512],
                   start=(kc == 0), stop=(kc == 1))
        nc.vector.tensor_scalar(out=kh_sb[:, mo, :], in0=ps,
                                scalar1=sb["bk"][:, mo, :], scalar2=None,
                                op0=OP.add)

    # q2 = wq @ qT + bq (wq pre-scaled by norm factor) -> qh_sb [128, 2, nq]
    qh_sb = cp.tile([128, 2, nq], F32, name="qh_sb")
    for mo in range(2):
        ps = pp.tile([128, 64], F32, tag="small", bufs=2)
        for kc in range(2):
            MM(ps[:, :nq], sb["wqT"][:, kc, mo, :], qT_sb[:, kc, :],
               start=(kc == 0), stop=(kc == 1))
        nc.vector.tensor_scalar(out=qh_sb[:, mo, :], in0=ps[:, :nq],
                                scalar1=sb["bq"][:, mo, :], scalar2=None,
                                op0=OP.add)

    # multiplicative 0/1 mask broadcast across partitions (via K=1 matmul)
    psm = pp.tile([128, S], F32, tag="big", bufs=3)
    for ncol in range(2):
        MM(psm[:, ncol * 512 : (ncol + 1) * 512],
           sb["ones"][0:1, :], msk_sb[0:1, ncol * 512 : (ncol + 1) * 512],
           start=True, stop=True)
    mb_sb = cp.tile([128, S], F32, name="mb_sb")
    nc.scalar.copy(out=mb_sb, in_=psm)

    # head matmuls: w[(n,q), s]; heads n=mo*4+j at partitions 32j of chunk mo
    exp_sb = []
    for mo in range(2):
        psw = pp.tile([128, S], F32, tag="big", bufs=3)
        nc.vector.memset(psw, 0.0)
        for j in range(4):
            pbase = 32 * j
            for ncol in range(2):
                MM(psw[pbase : pbase + nq, ncol * 512 : (ncol + 1) * 512],
                   qh_sb[pbase : pbase + 32, mo, :],
                   kh_sb[pbase : pbase + 32, mo, ncol * 512 : (ncol + 1) * 512],
                   start=True, stop=True,
                   tile_position=(pbase, pbase))
        e = cp.tile([128, S], F32, name=f"exp_sb{mo}")
        nc.scalar.activation(out=e, in_=psw, func=AF.Exp)
        # masked positions contribute exp(w)*0
        nc.vector.tensor_mul(e, e, mb_sb)
        exp_sb.append(e)

    # ws[q, s] = sum_n exp; normalize by 1/Z
    psws = pp.tile([128, S], F32, tag="big", bufs=3)
    for ncol in range(2):
        for mo in range(2):
            MM(psws[:nq, ncol * 512 : (ncol + 1) * 512],
               sb["gq"], exp_sb[mo][:, ncol * 512 : (ncol + 1) * 512],
               start=(mo == 0), stop=(mo == 1))
    z = tp.tile([32, 1], F32, tag="z")
    nc.vector.reduce_sum(out=z[:nq], in_=psws[:nq, :], axis=mybir.AxisListType.X)
    rz = tp.tile([32, 1], F32, tag="rz")
    nc.vector.reciprocal(out=rz[:nq], in_=z[:nq])
    ws_sb = cp.tile([32, S], F32, name="ws_sb")
    nc.vector.tensor_scalar(out=ws_sb[:nq], in0=psws[:nq, :], scalar1=rz[:nq],
                            scalar2=None, op0=OP.mult)

    # wsT [s, q] via PE transposes; ws2T = wsT^2
    wsT = cp.tile([128, 8, 32], F32, name="wsT")
    ws2T = cp.tile([128, 8, 32], F32, name="ws2T")
    for sc in range(8):
        pst = pp.tile([128, 64], F32, tag="small", bufs=2)
        nc.tensor.transpose(pst[:, :nq], ws_sb[:nq, sc * 128 : (sc + 1) * 128],
                            sb["ident"][:nq, :nq])
        nc.scalar.copy(out=wsT[:, sc, :nq], in_=pst[:, :nq])
    nc.scalar.square(out=ws2T[:, :, :nq], in_=wsT[:, :, :nq])



    # ================= hoisted 1x1 convs =================
    # Y_j = conv1x1_j(x), plus group colsums S_jT/Q_jT [s, g] (scaled 1/16384)
    y_sb = {}
    sq_sb = {}
    for j, wname in (("0", "w0T"), ("1", "w1c0T"), ("r", "wresT")):
        psy = pp.tile([128, S], F32, tag="big", bufs=3)
        for kc in range(4):
            for ncol in range(2):
                MM(psy[:, ncol * 512 : (ncol + 1) * 512],
                   sb[wname][:, kc, 0, :],
                   x_sb[:, kc, ncol * 512 : (ncol + 1) * 512],
                   start=(kc == 0), stop=(kc == 3))
        y = cp.tile([128, S], F32, name=f"y_sb{j}")
        nc.scalar.copy(out=y, in_=psy)
        y_sb[j] = y
        ysq = qp.tile([128, S], F32, tag="ysq", bufs=2)
        nc.scalar.square(out=ysq, in_=y)
        # S_jT[s, g] = sum_{c in g} Y[c, s]/16384 ; Q from Y^2
        sq = cp.tile([128, 8, 16], F32, name=f"sq_sb{j}")
        for sc in range(8):
            pss = pp.tile([128, 64], F32, tag="small", bufs=2)
            MM(pss[:, 0:8], y[:, sc * 128 : (sc + 1) * 128], sb["g8s"],
               start=True, stop=True)
            MM(pss[:, 8:16], ysq[:, sc * 128 : (sc + 1) * 128], sb["g8s"],
               start=True, stop=True)
            nc.scalar.copy(out=sq[:, sc, :], in_=pss[:, 0:16])
        sq_sb[j] = sq

    # per-query GN scale/bias for x0/x1a/xr: [128, nq]
    GCOL = {"0": 0, "1": 1, "r": 6}
    scale_pre = {}
    bias_pre = {}
    for j in ("0", "1", "r"):
        pst = pp.tile([8, 64], F32, tag="small", bufs=2)
        for sc in range(8):
            MM(pst[:8, 0:nq], sq_sb[j][:, sc, 0:8], wsT[:, sc, :nq],
               start=(sc == 0), stop=(sc == 7))
        for sc in range(8):
            MM(pst[:8, 32 : 32 + nq], sq_sb[j][:, sc, 8:16], ws2T[:, sc, :nq],
               start=(sc == 0), stop=(sc == 7))
        st = tp.tile([8, 64], F32, tag="st8")
        nc.scalar.copy(out=st[:, 0:nq], in_=pst[:8, 0:nq])
        nc.scalar.copy(out=st[:, 32 : 32 + nq], in_=pst[:8, 32 : 32 + nq])
        m2 = tp.tile([8, 32], F32, tag="m28")
        nc.scalar.square(out=m2[:, :nq], in_=st[:, 0:nq])
        var = tp.tile([8, 32], F32, tag="var8")
        nc.vector.tensor_sub(var[:, :nq], st[:, 32 : 32 + nq], m2[:, :nq])
        sd = tp.tile([8, 32], F32, tag="sd8")
        nc.scalar.activation(out=sd[:, :nq], in_=var[:, :nq], func=AF.Sqrt,
                             bias=sb["eps8"], scale=1.0)
        rstd = tp.tile([8, 32], F32, tag="rstd8")
        nc.vector.reciprocal(out=rstd[:, :nq], in_=sd[:, :nq])
        # broadcast to channels
        pbc = pp.tile([128, 64], F32, tag="small", bufs=2)
        MM(pbc[:, 0:nq], sb["g8bc"], rstd[:8, :nq], start=True, stop=True)
        MM(pbc[:, 32 : 32 + nq], sb["g8bcn"], st[:8, 0:nq], start=True, stop=True)
        gcol = GCOL[j]
        sc_t = cp.tile([128, 32], F32, name=f"scale_pre{j}")
        nc.vector.tensor_scalar(out=sc_t[:, :nq], in0=pbc[:, 0:nq],
                                scalar1=sb["gn_g"][:, gcol : gcol + 1],
                                scalar2=None, op0=OP.mult)
        tmp = tp.tile([128, 32], F32, tag="bptmp")
        nc.vector.tensor_mul(tmp[:, :nq], pbc[:, 32 : 32 + nq], sc_t[:, :nq])
        bi_t = cp.tile([128, 32], F32, name=f"bias_pre{j}")
        nc.vector.tensor_scalar(out=bi_t[:, :nq], in0=tmp[:, :nq],
                                scalar1=sb["gn_b"][:, gcol : gcol + 1],
                                scalar2=None, op0=OP.add)
        scale_pre[j] = sc_t
        bias_pre[j] = bi_t

    # ================= helpers =================
    def conv_taps(psum_flat, srcs_w, taps):
        """Accumulate conv into psum [128, 1024] viewed [128, 32, 32].
        srcs_w: list of (src_tile_flat, weight_ap_fn(tap_idx) -> [128,128]).
        taps: list of (dh, dw), center (0,0) must be included."""
        p3 = psum_flat.rearrange("p (h w) -> p h w", w=HW)
        order = sorted(range(len(taps)), key=lambda i: taps[i] != (0, 0))
        for bank in range(2):
            ops = []
            for oi in order:
                dh, dw = taps[oi]
                h0, h1 = max(0, -dh), HW - max(0, dh)
                w0, w1 = max(0, -dw), HW - max(0, dw)
                bh0, bh1 = max(h0, bank * 16), min(h1, bank * 16 + 16)
                if bh0 >= bh1:
                    continue
                for src, wfn in srcs_w:
                    s3 = src.rearrange("p (h w) -> p h w", w=HW)
                    ops.append((p3[:, bh0:bh1, w0:w1], wfn(oi),
                                s3[:, bh0 + dh : bh1 + dh, w0 + dw : w1 + dw]))
            for i, (o, w, r) in enumerate(ops):
                MM(o, w, r, start=(i == 0), stop=(i == len(ops) - 1),
                   skip_group_check=True)

    def gn_stats(psum_flat, gcol):
        """GroupNorm scale/bias [128,1] from conv output in PSUM."""
        st6 = tp.tile([128, 2, 6], F32, tag="st6")
        nc.vector.bn_stats(out=st6[:, 0, :], in_=psum_flat[:, 0:512])
        nc.vector.bn_stats(out=st6[:, 1, :], in_=psum_flat[:, 512:1024])
        mv = tp.tile([128, 2], F32, tag="mv")
        nc.vector.bn_aggr(out=mv, in_=st6)
        # mv = [mean_p, var_p] -> [mean_p, E[x^2]_p]
        m2 = tp.tile([128, 1], F32, tag="m2")
        nc.scalar.square(out=m2, in_=mv[:, 0:1])
        nc.vector.tensor_add(mv[:, 1:2], mv[:, 1:2], m2)
        # group-mean + broadcast to every partition in the group via G128
        pst = pp.tile([128, 64], F32, tag="small", bufs=2)
        MM(pst[:, 0:2], sb["g128s"], mv, start=True, stop=True)
        # pst = [group mean, group E[x^2]] broadcast per partition
        gm2 = tp.tile([128, 1], F32, tag="gm2")
        nc.scalar.square(out=gm2, in_=pst[:, 0:1])
        var = tp.tile([128, 1], F32, tag="var")
        nc.vector.tensor_sub(var, pst[:, 1:2], gm2)
        sd = tp.tile([128, 1], F32, tag="sd")
        nc.scalar.activation(out=sd, in_=var, func=AF.Sqrt, bias=sb["eps128"],
                             scale=1.0)
        rstd = tp.tile([128, 1], F32, tag="rstd")
        nc.vector.reciprocal(out=rstd, in_=sd)
        scale = tp.tile([128, 1], F32, tag="scale")
        nc.vector.tensor_mul(scale, sb["gn_g"][:, gcol : gcol + 1], rstd)
        mt = tp.tile([128, 1], F32, tag="mt")
        nc.vector.tensor_mul(mt, pst[:, 0:1], scale)
        biasv = tp.tile([128, 1], F32, tag="biasv")
        nc.vector.tensor_sub(biasv, sb["gn_b"][:, gcol : gcol + 1], mt)
        return scale, biasv

    TAPS_1x3 = [(0, -1), (0, 0), (0, 1)]
    TAPS_3x1 = [(-1, 0), (0, 0), (1, 0)]
    TAPS_3x3d3 = [(dh, dw) for dh in (-3, 0, 3) for dw in (-3, 0, 3)]
    TAPS_3x3 = [(dh, dw) for dh in (-1, 0, 1) for dw in (-1, 0, 1)]

    # ================= per-query main loop =================
    for q in range(nq):
        # ws row for this query (partition 0 of a ring slot), then
        # broadcast to all 128 partitions via a K=1 matmul
        wrow = qp.tile([1, S], F32, tag="wsrow", bufs=4)
        nc.sync.dma_start(out=wrow, in_=ws_sb[q : q + 1, :])
        pwsb = pp.tile([128, S], F32, tag="big", bufs=3)
        for ncol in range(2):
            MM(pwsb[:, ncol * 512 : (ncol + 1) * 512],
               sb["ones"][0:1, :],
               wrow[0:1, ncol * 512 : (ncol + 1) * 512],
               start=True, stop=True)
        wsb = qp.tile([128, S], F32, tag="wsb_sb", bufs=2)
        nc.scalar.copy(out=wsb, in_=pwsb)

        # x0, x1a, tr
        t0 = qp.tile([128, S], F32, tag="t0", bufs=2)
        nc.vector.tensor_mul(t0, y_sb["0"], wsb)
        x0 = qp.tile([128, S], F32, tag="x0", bufs=2)
        nc.scalar.activation(out=x0, in_=t0, func=AF.Identity,
                             bias=bias_pre["0"][:, q : q + 1],
                             scale=scale_pre["0"][:, q : q + 1])
        t1 = qp.tile([128, S], F32, tag="t1", bufs=2)
        nc.gpsimd.tensor_mul(t1, y_sb["1"], wsb)
        x1a = qp.tile([128, S], F32, tag="xA", bufs=2)
        nc.scalar.activation(out=x1a, in_=t1, func=AF.Identity,
                             bias=bias_pre["1"][:, q : q + 1],
                             scale=scale_pre["1"][:, q : q + 1])
        tr = qp.tile([128, S], F32, tag="tr", bufs=2)
        nc.vector.tensor_mul(tr, y_sb["r"], wsb)

        # L1: b1c1 (1x3)
        ps1 = pp.tile([128, S], F32, tag="big", bufs=3)
        conv_taps(ps1, [(x1a, lambda t: sb["w1c1T"][:, 0, t, :])], TAPS_1x3)
        s1, b1 = gn_stats(ps1, 2)
        x1b = qp.tile([128, S], F32, tag="xB", bufs=2)
        nc.scalar.activation(out=x1b, in_=ps1, func=AF.Identity, bias=b1, scale=s1)

        # L2: b1c2 (3x1)
        ps2 = pp.tile([128, S], F32, tag="big", bufs=3)
        conv_taps(ps2, [(x1b, lambda t: sb["w1c2T"][:, 0, t, :])], TAPS_3x1)
        s2, b2 = gn_stats(ps2, 3)
        x1c = qp.tile([128, S], F32, tag="xA", bufs=2)
        nc.scalar.activation(out=x1c, in_=ps2, func=AF.Identity, bias=b2, scale=s2)

        # L3: b1c3 (3x3 dil 3)
        ps3 = pp.tile([128, S], F32, tag="big", bufs=3)
        conv_taps(ps3, [(x1c, lambda t: sb["w1c3T"][:, 0, t, :])], TAPS_3x3d3)
        s3, b3 = gn_stats(ps3, 4)
        x1d = qp.tile([128, S], F32, tag="xB", bufs=2)
        nc.scalar.activation(out=x1d, in_=ps3, func=AF.Identity, bias=b3, scale=s3)

        # L4: cat conv (3x3 on [x0; x1d])
        ps4 = pp.tile([128, S], F32, tag="big", bufs=3)
        conv_taps(ps4, [(x0, lambda t: sb["wcatT"][:, 0, t, :]),
                        (x1d, lambda t: sb["wcatT"][:, 1, t, :])], TAPS_3x3)
        s4, b4 = gn_stats(ps4, 5)

        # xr with bias_cat folded in; final relu(xc + xr)
        brt = tp.tile([128, 1], F32, tag="brt")
        nc.vector.tensor_scalar(out=brt, in0=b4,
                                scalar1=bias_pre["r"][:, q : q + 1],
                                scalar2=None, op0=OP.add)
        xr = qp.tile([128, S], F32, tag="t0", bufs=2)
        nc.scalar.activation(out=xr, in_=tr, func=AF.Identity, bias=brt,
                             scale=scale_pre["r"][:, q : q + 1])
        u = qp.tile([128, S], F32, tag="u", bufs=2)
        nc.vector.scalar_tensor_tensor(out=u, in0=ps4, scalar=s4, in1=xr,
                                       op0=OP.mult, op1=OP.add)
        o = qp.tile([128, S], F32, tag="o", bufs=2)
        nc.gpsimd.tensor_relu(o, u)
        nc.sync.dma_start(out=out_h[...][q], in_=o)

    ctx.close()


# ----------------------------------------------------------------------------
# public entry point
# ----------------------------------------------------------------------------

_NC_CACHE = {}


def _get_program(nq=NQ):
    if nq not in _NC_CACHE:
        _NC_CACHE[nq] = build_program(nq)
    return _NC_CACHE[nq]


def make_in_maps(x, q, k, mask, params, nq=NQ):
    consts = _prep_consts(params)
    consts["gq"] = np.ascontiguousarray(consts["gq"][:, :nq])
    x = np.asarray(x, np.float32)
    q = np.asarray(q, np.float32)
    k = np.asarray(k, np.float32)
    mask = np.asarray(mask)
    in_maps = []
    for core in range(NCORES):
        b = core // (NCORES // B)
        q0 = (core % (NCORES // B)) * NQ  # stride 25 regardless of debug nq
        m = dict(consts)
        m["xk"] = np.ascontiguousarray(x[b].reshape(INC, S))
        m["kk"] = np.ascontiguousarray(k[b].reshape(QD, S))
        m["qT"] = np.ascontiguousarray(q[b, q0 : q0 + nq].T)
        m["mrow"] = np.ascontiguousarray(
            np.where(mask[b].reshape(1, S), np.float32(0.0), np.float32(1.0))
        ).astype(np.float32)
        in_maps.append(m)
    return in_maps


def kernel(x, q, k, mask, params):
    nc = _get_program(NQ)
    in_maps = make_in_maps(x, q, k, mask, params, NQ)
    res = run_bass_kernel_spmd(nc, in_maps, core_ids=list(range(NCORES)))
    outs = [res.results[i]["out"] for i in range(NCORES)]
    full = np.concatenate(outs, axis=0)  # [200, 128, 1024]
    return full.reshape(B * Q, OC, HW, HW)


if __name__ == "__main__":
    import reference

    inputs = reference.setup_inputs()
    out = kernel(**{k_: np.asarray(v) if not isinstance(v, dict) else v
                    for k_, v in inputs.items()})
    print("kernel out:", out.shape, out.dtype)


# revision 22
# speedup vs baseline: 3.3935x; 3.3935x over previous
"""Trainium2 Bass kernel for nn_CrossFusionModule (sparse_attention).

Computation (per batch b, query q):
  q2 = q @ wq.T + bq ; k2 = wk*k + bk  (1x1 conv)
  w[q,n,s] = (q2/sqrt(dh))[q,n,:] . k2[n,:,s] ; softmax over (n,s); ws = sum_n
  xf = x * ws[q]   (per-pixel scale of the [512,1024] feature map)
  RefineModule: x0 = GN(conv1x1_b0(xf)); x1 = chain of convs+GN;
  xc = GN(conv3x3_cat([x0,x1])); xr = GN(conv1x1_res(xf)); out = relu(xc+xr)

Sharding: 8 cores; core i handles b = i//4 and 25 queries (i%4)*25..+25.
Key trick: conv1x1(x*ws) = conv1x1(x)*ws, so Y_j = conv1x1_j(x) is computed
once per core and only scaled per query; their GN stats are computed with
tiny matmuls against ws and ws^2 (group colsums hoisted per batch).
"""

import sys
import types

import numpy as np

for p in ("/opt/trn_rl_repo", "/root/.axon_site"):
    if p not in sys.path:
        sys.path.insert(0, p)

import concourse.bass as bass
import concourse.tile as tile
from concourse import mybir
from concourse.bass_utils import run_bass_kernel_spmd
from concourse.vector_clock import ScopedClock

F32 = mybir.dt.float32
F16 = mybir.dt.float16
AF = mybir.ActivationFunctionType
OP = mybir.AluOpType

B, Q, QD, HD, NH, HW = 2, 100, 256, 256, 8, 32
S = HW * HW  # 1024
CH = 64
OC = 2 * CH  # 128
INC = 512
GN_GROUPS = 8
EPS = 1e-5
NCORES = 8
NQ = (B * Q) // NCORES  # 25 queries per core
GSZ = OC // GN_GROUPS  # 16 channels per group
NG_INV = 1.0 / (GSZ * S)  # 1/16384


def _patch_tile_drain():
    """Work around walrus 'Too many sync wait commands' on the kernel-tail
    Drain: spill its semaphore waits onto individual SP nops."""

    def _drain_and_barrier(self, tick_clock, wait_clock):
        drain_inst = self.nc.sync.drain()
        wait_clock.add_sem_waits(
            drain_inst.ins, ScopedClock({None: tick_clock.global_clock})
        )
        mi = drain_inst.ins
        waits = list(mi.sync_info.on_wait) if mi.sync_info is not None else []
        if len(waits) > 1:
            del mi.sync_info.on_wait[:]
            bb = self.nc.cur_bb.bb
            bb.instructions.pop()
            for w in waits:
                nop = self.nc.sync.nop(nofuse=True, hint="drain_wait_spill")
                if nop.ins.sync_info is None:
                    nop.ins.sync_info = mybir.SyncInfo(on_wait=[w], on_update=[])
                else:
                    nop.ins.sync_info.on_wait.append(w)
            bb.instructions.append(mi)
        self.nc.all_engine_barrier()
        popped = self.nc._tile_sem_poison_stack.pop()
        assert popped is self._sem_poison
        self.nc.clear_and_free_semaphores(list(self.sems.allocated().values()))
        self.nc.all_engine_barrier()

    tile.TileContext._drain_and_barrier = _drain_and_barrier


def _register_ntff_hook():
    """The trimmed container lacks antenv.axon_hooks; register the NTFF
    profiling hook manually so trace=True works (used by test.py only)."""
    if "antenv.axon_hooks" in sys.modules:
        return
    try:
        from trn_agent_boot.trn_boot import _ntff_profile_via_ctypes

        hook = _ntff_profile_via_ctypes("/opt/axon/libaxon_pjrt.so")
    except Exception:
        hook = None
    mod = types.ModuleType("antenv.axon_hooks")
    mod.get_axon_ntff_profile_hook = lambda: hook
    mod.set_axon_ntff_profile_hook = lambda h: None
    sys.modules["antenv.axon_hooks"] = mod


_patch_tile_drain()
_register_ntff_hook()

_DMA_INSTS = tuple(
    getattr(mybir, n)
    for n in ("InstDMACopy", "InstDmaTransposeAnt", "InstDMAGatherAnt",
              "InstDMAScatterAddAnt", "InstCollectiveCompute")
    if hasattr(mybir, n)
)


def _spill_waits(nc, maxw=1):
    """This walrus build supports only one semaphore wait per engine
    instruction and does not auto-split; move excess waits onto nops."""
    nid = 0
    for f in nc.m.functions:
        for blk in f.blocks:
            new = []
            for inst in blk.instructions:
                si = getattr(inst, "sync_info", None)
                if si is not None and si.on_wait and len(si.on_wait) > maxw:
                    waits = list(si.on_wait)
                    del si.on_wait[:]
                    si.on_wait.extend(waits[-maxw:])
                    for w in waits[:-maxw]:
                        nop = mybir.InstNoOp(
                            name=f"I-wspill-{nid}",
                            engine=inst.engine,
                            sync_info=mybir.SyncInfo(on_wait=[w], on_update=[]),
                            bass_nofuse=True,
                        )
                        nid += 1
                        nc.register_instruction(nop)
                        new.append(nop)
                new.append(inst)
            blk.instructions[:] = new


# ----------------------------------------------------------------------------
# host-side weight prep
# ----------------------------------------------------------------------------

def _prep_consts(params):
    """Transform weights into the SBUF layouts the kernel consumes."""
    nf = float(HD / NH) ** (-0.5)
    c = {}

    def npf(a):
        return np.ascontiguousarray(np.asarray(a, dtype=np.float32))

    # wq/wk: [HD, QD] -> [kc, p, mo, o]: w[mo*128+o, kc*128+p]
    for name, w, bvec, scale in (
        ("wq", params["wq"], params["bq"], nf),
        ("wk", params["wk"], params["bk"], 1.0),
    ):
        wt = npf(w) * scale  # [256 o, 256 c]
        # [kc, p, mo, o] -> [p, kc, mo, o]
        c[name + "T"] = npf(wt.T.reshape(2, 128, 2, 128).transpose(1, 0, 2, 3))
        bv = npf(np.asarray(bvec, np.float32) * scale).reshape(2, 128)
        c["b" + name[1]] = npf(bv.T[:, :, None])  # [o(p), mo, 1]

    # conv weights: w [O, I, kh, kw] -> WT [p, kc, tap, o] with c = kc*128+p
    def convT(p):
        w = npf(p["w"])  # [O, I, kh, kw]
        O_, I_, kh, kw = w.shape
        kc = I_ // 128
        # [p, kc, tap, o]
        wt = w.reshape(O_, kc, 128, kh * kw).transpose(2, 1, 3, 0)
        return npf(wt)

    c["w0T"] = convT(params["b0"])      # [128, 4, 1, 128]
    c["w1c0T"] = convT(params["b1c0"])  # [128, 4, 1, 128]
    c["wresT"] = convT(params["res"])   # [128, 4, 1, 128]
    c["w1c1T"] = convT(params["b1c1"]).astype(np.float16)  # [128, 1, 3, 128]
    c["w1c2T"] = convT(params["b1c2"]).astype(np.float16)  # [128, 1, 3, 128]
    c["w1c3T"] = convT(params["b1c3"]).astype(np.float16)  # [128, 1, 9, 128]
    c["wcatT"] = convT(params["cat"]).astype(np.float16)   # [128, 2, 9, 128]

    # GN gamma/beta columns: [128, 7]; order: b0,b1c0,b1c1,b1c2,b1c3,cat,res
    order = ["b0", "b1c0", "b1c1", "b1c2", "b1c3", "cat", "res"]
    c["gn_g"] = npf(np.stack([npf(params[k]["g"]) for k in order], axis=1))
    c["gn_b"] = npf(np.stack([npf(params[k]["b"]) for k in order], axis=1))

    # constants
    c["ident"] = npf(np.eye(128, dtype=np.float32))
    c["ones"] = npf(np.ones((128, 128), np.float32))
    g8 = np.zeros((128, 8), np.float32)
    for i in range(128):
        g8[i, i // GSZ] = NG_INV
    c["g8s"] = npf(g8)  # scaled group indicator (for colsums -> means)
    g8bc = np.zeros((8, 128), np.float32)
    for i in range(128):
        g8bc[i // GSZ, i] = 1.0
    c["g8bc"] = npf(g8bc)
    c["g8bcn"] = npf(-g8bc)
    g128 = np.zeros((128, 128), np.float32)
    for i in range(128):
        g0 = (i // GSZ) * GSZ
        g128[g0 : g0 + GSZ, i] = 1.0 / GSZ
    # [c, p]: out[p] = mean over group(p) of per-partition stats (1/16 each)
    c["g128s"] = npf(g128)
    gq = np.zeros((128, NQ), np.float32)
    for j in range(4):
        for i in range(NQ):
            gq[32 * j + i, i] = 1.0
    c["gq"] = npf(gq)
    c["eps128"] = npf(np.full((128, 1), EPS, np.float32))
    c["eps8"] = npf(np.full((8, 1), EPS, np.float32))
    return c


CONST_NAMES = [
    "wqT", "bq", "wkT", "bk",
    "w0T", "w1c0T", "wresT", "w1c1T", "w1c2T", "w1c3T", "wcatT",
    "gn_g", "gn_b",
    "ident", "ones", "g8s", "g8bc", "g8bcn", "g128s", "gq", "eps128", "eps8",
]


# ----------------------------------------------------------------------------
# device program
# ----------------------------------------------------------------------------

def build_program(nq=NQ):
    nc = bass.Bass("TRN2", target_bir_lowering=False)

    # ---- DRAM I/O ----
    d = {}
    d["xk"] = nc.dram_tensor("xk", [INC, S], F32, kind="ExternalInput")
    d["kk"] = nc.dram_tensor("kk", [QD, S], F32, kind="ExternalInput")
    d["qT"] = nc.dram_tensor("qT", [QD, nq], F32, kind="ExternalInput")
    d["mrow"] = nc.dram_tensor("mrow", [1, S], F32, kind="ExternalInput")
    shapes = {
        "wqT": [128, 2, 2, 128], "bq": [128, 2, 1],
        "wkT": [128, 2, 2, 128], "bk": [128, 2, 1],
        "w0T": [128, 4, 1, 128], "w1c0T": [128, 4, 1, 128],
        "wresT": [128, 4, 1, 128],
        "w1c1T": [128, 1, 3, 128], "w1c2T": [128, 1, 3, 128],
        "w1c3T": [128, 1, 9, 128], "wcatT": [128, 2, 9, 128],
        "gn_g": [128, 7], "gn_b": [128, 7],
        "ident": [128, 128], "ones": [128, 128],
        "g8s": [128, 8], "g8bc": [8, 128], "g8bcn": [8, 128],
        "g128s": [128, 128], "gq": [128, nq],
        "eps128": [128, 1], "eps8": [8, 1],
    }
    F16_CONSTS = {"w1c1T", "w1c2T", "w1c3T", "wcatT"}
    for n in CONST_NAMES:
        dt = F16 if n in F16_CONSTS else F32
        d[n] = nc.dram_tensor(n, shapes[n], dt, kind="ExternalInput")
    out_h = nc.dram_tensor("out", [nq, OC, S], F32, kind="ExternalOutput")

    with tile.TileContext(nc) as tc:
        _emit(nc, tc, d, out_h, nq)
    _spill_waits(nc)
    return nc


def _emit(nc, tc, d, out_h, nq):
    import contextlib

    ctx = contextlib.ExitStack()
    cp = ctx.enter_context(tc.tile_pool(name="const", bufs=1))
    qp = ctx.enter_context(tc.tile_pool(name="work", bufs=2))
    tp = ctx.enter_context(tc.tile_pool(name="tiny", bufs=4))
    pp = ctx.enter_context(tc.tile_pool(name="psum", bufs=1, space="PSUM"))

    # ---- load constants ----
    sb = {}
    for n in ("wqT", "bq", "wkT", "bk", "w0T", "w1c0T", "wresT", "w1c1T",
              "w1c2T", "w1c3T", "wcatT", "gn_g", "gn_b", "ident", "ones",
              "g8s", "g8bc", "g8bcn", "g128s", "gq", "eps128", "eps8"):
        t = cp.tile(list(d[n].shape), d[n].dtype, name="sb_" + n)
        nc.sync.dma_start(out=t, in_=d[n][...])
        sb[n] = t

    # inputs (chunk tiles aliased onto main-loop work tags to save SBUF)
    xch = []
    for i in range(4):
        t = qp.tile([128, S], F32, tag="tr", bufs=6, name=f"xch{i}")
        nc.sync.dma_start(out=t, in_=d["xk"][:][i * 128 : (i + 1) * 128, :])
        xch.append(t)
    kch = []
    for i in range(2):
        t = qp.tile([128, S], F32, tag=("t0", "t1")[i], bufs=3, name=f"kch{i}")
        nc.sync.dma_start(out=t, in_=d["kk"][:][i * 128 : (i + 1) * 128, :])
        kch.append(t)
    qT_sb = cp.tile([128, 2, nq], F32, name="qT_sb")
    nc.sync.dma_start(out=qT_sb, in_=d["qT"][...].rearrange("(kc p) q -> p kc q", p=128))
    msk_sb = cp.tile([1, S], F32, name="msk_sb")
    nc.sync.dma_start(out=msk_sb, in_=d["mrow"][...])

    MM = nc.tensor.matmul

    # ================= attention preamble =================
    # k2 = wk @ k + bk -> kh[mo] [128, 1024]
    kh = []
    for mo in range(2):
        ps = pp.tile([128, S], F32, tag="big", bufs=3)
        for kc in range(2):
            for ncol in range(2):
                MM(ps[:, ncol * 512 : (ncol + 1) * 512],
                   sb["wkT"][:, kc, mo, :],
                   kch[kc][:, ncol * 512 : (ncol + 1) * 512],
                   start=(kc == 0), stop=(kc == 1))
        t = qp.tile([128, S], F32, tag=("u", "xr")[mo], bufs=3, name=f"kh{mo}")
        nc.vector.tensor_scalar(out=t, in0=ps,
                                scalar1=sb["bk"][:, mo, :], scalar2=None,
                                op0=OP.add)
        kh.append(t)

    # q2 = wq @ qT + bq (wq pre-scaled by norm factor) -> qh_sb [128, 2, nq]
    qh_sb = cp.tile([128, 2, nq], F32, name="qh_sb")
    for mo in range(2):
        ps = pp.tile([128, 64], F32, tag="small", bufs=2)
        for kc in range(2):
            MM(ps[:, :nq], sb["wqT"][:, kc, mo, :], qT_sb[:, kc, :],
               start=(kc == 0), stop=(kc == 1))
        nc.vector.tensor_scalar(out=qh_sb[:, mo, :], in0=ps[:, :nq],
                                scalar1=sb["bq"][:, mo, :], scalar2=None,
                                op0=OP.add)

    # multiplicative 0/1 mask broadcast across partitions (via K=1 matmul)
    psm = pp.tile([128, S], F32, tag="big", bufs=3)
    for ncol in range(2):
        MM(psm[:, ncol * 512 : (ncol + 1) * 512],
           sb["ones"][0:1, :], msk_sb[0:1, ncol * 512 : (ncol + 1) * 512],
           start=True, stop=True)
    mb_sb = cp.tile([128, S], F32, name="mb_sb")
    nc.scalar.copy(out=mb_sb, in_=psm)

    # head matmuls: w[(n,q), s]; heads n=mo*4+j at partitions 32j of chunk mo
    exp_sb = []
    for mo in range(2):
        psw = pp.tile([128, S], F32, tag="big", bufs=3)
        nc.vector.memset(psw, 0.0)
        for j in range(4):
            pbase = 32 * j
            for ncol in range(2):
                MM(psw[pbase : pbase + nq, ncol * 512 : (ncol + 1) * 512],
                   qh_sb[pbase : pbase + 32, mo, :],
                   kh[mo][pbase : pbase + 32, ncol * 512 : (ncol + 1) * 512],
                   start=True, stop=True,
                   tile_position=(pbase, pbase))
        e = qp.tile([128, S], F32, tag="wsb_sb", bufs=3, name=f"exp_sb{mo}")
        nc.scalar.activation(out=e, in_=psw, func=AF.Exp)
        # masked positions contribute exp(w)*0
        nc.vector.tensor_mul(e, e, mb_sb)
        exp_sb.append(e)

    # ws[q, s] = sum_n exp; normalize by 1/Z
    psws = pp.tile([128, S], F32, tag="big", bufs=3)
    for ncol in range(2):
        for mo in range(2):
            MM(psws[:nq, ncol * 512 : (ncol + 1) * 512],
               sb["gq"], exp_sb[mo][:, ncol * 512 : (ncol + 1) * 512],
               start=(mo == 0), stop=(mo == 1))
    z = tp.tile([32, 1], F32, tag="z")
    nc.vector.reduce_sum(out=z[:nq], in_=psws[:nq, :], axis=mybir.AxisListType.X)
    rz = tp.tile([32, 1], F32, tag="rz")
    nc.vector.reciprocal(out=rz[:nq], in_=z[:nq])
    ws_sb = cp.tile([32, S], F32, name="ws_sb")
    nc.vector.tensor_scalar(out=ws_sb[:nq], in0=psws[:nq, :], scalar1=rz[:nq],
                            scalar2=None, op0=OP.mult)

    # wsT [s, q] via PE transposes; ws2T = wsT^2
    wsT = cp.tile([128, 8, 32], F32, name="wsT")
    ws2T = cp.tile([128, 8, 32], F32, name="ws2T")
    for sc in range(8):
        pst = pp.tile([128, 64], F32, tag="small", bufs=2)
        nc.tensor.transpose(pst[:, :nq], ws_sb[:nq, sc * 128 : (sc + 1) * 128],
                            sb["ident"][:nq, :nq])
        nc.scalar.copy(out=wsT[:, sc, :nq], in_=pst[:, :nq])
    nc.scalar.square(out=ws2T[:, :, :nq], in_=wsT[:, :, :nq])



    # ================= hoisted 1x1 convs =================
    # Y_j = conv1x1_j(x), plus group colsums S_jT/Q_jT [s, g] (scaled 1/16384)
    y_sb = {}
    sq_sb = {}
    for j, wname in (("0", "w0T"), ("1", "w1c0T"), ("r", "wresT")):
        psy = pp.tile([128, S], F32, tag="big", bufs=3)
        for kc in range(4):
            for ncol in range(2):
                MM(psy[:, ncol * 512 : (ncol + 1) * 512],
                   sb[wname][:, kc, 0, :],
                   xch[kc][:, ncol * 512 : (ncol + 1) * 512],
                   start=(kc == 0), stop=(kc == 3))
        y = cp.tile([128, S], F32, name=f"y_sb{j}")
        nc.scalar.copy(out=y, in_=psy)
        y_sb[j] = y
        ysq = qp.tile([128, S], F32, tag="u", bufs=3)
        nc.scalar.square(out=ysq, in_=y)
        # S_jT[s, g] = sum_{c in g} Y[c, s]/16384 ; Q from Y^2
        sq = cp.tile([128, 8, 16], F32, name=f"sq_sb{j}")
        for sc in range(8):
            pss = pp.tile([128, 64], F32, tag="small", bufs=2)
            MM(pss[:, 0:8], y[:, sc * 128 : (sc + 1) * 128], sb["g8s"],
               start=True, stop=True)
            MM(pss[:, 8:16], ysq[:, sc * 128 : (sc + 1) * 128], sb["g8s"],
               start=True, stop=True)
            nc.scalar.copy(out=sq[:, sc, :], in_=pss[:, 0:16])
        sq_sb[j] = sq

    # per-query GN scale/bias for x0/x1a/xr: [128, nq]
    GCOL = {"0": 0, "1": 1, "r": 6}
    scale_pre = {}
    bias_pre = {}
    for j in ("0", "1", "r"):
        pst = pp.tile([8, 64], F32, tag="small", bufs=2)
        for sc in range(8):
            MM(pst[:8, 0:nq], sq_sb[j][:, sc, 0:8], wsT[:, sc, :nq],
               start=(sc == 0), stop=(sc == 7))
        for sc in range(8):
            MM(pst[:8, 32 : 32 + nq], sq_sb[j][:, sc, 8:16], ws2T[:, sc, :nq],
               start=(sc == 0), stop=(sc == 7))
        st = tp.tile([8, 64], F32, tag="st8")
        nc.scalar.copy(out=st[:, 0:nq], in_=pst[:8, 0:nq])
        nc.scalar.copy(out=st[:, 32 : 32 + nq], in_=pst[:8, 32 : 32 + nq])
        m2 = tp.tile([8, 32], F32, tag="m28")
        nc.scalar.square(out=m2[:, :nq], in_=st[:, 0:nq])
        var = tp.tile([8, 32], F32, tag="var8")
        nc.vector.tensor_sub(var[:, :nq], st[:, 32 : 32 + nq], m2[:, :nq])
        sd = tp.tile([8, 32], F32, tag="sd8")
        nc.scalar.activation(out=sd[:, :nq], in_=var[:, :nq], func=AF.Sqrt,
                             bias=sb["eps8"], scale=1.0)
        rstd = tp.tile([8, 32], F32, tag="rstd8")
        nc.vector.reciprocal(out=rstd[:, :nq], in_=sd[:, :nq])
        # broadcast to channels
        pbc = pp.tile([128, 64], F32, tag="small", bufs=2)
        MM(pbc[:, 0:nq], sb["g8bc"], rstd[:8, :nq], start=True, stop=True)
        MM(pbc[:, 32 : 32 + nq], sb["g8bcn"], st[:8, 0:nq], start=True, stop=True)
        gcol = GCOL[j]
        sc_t = cp.tile([128, 32], F32, name=f"scale_pre{j}")
        nc.vector.tensor_scalar(out=sc_t[:, :nq], in0=pbc[:, 0:nq],
                                scalar1=sb["gn_g"][:, gcol : gcol + 1],
                                scalar2=None, op0=OP.mult)
        tmp = tp.tile([128, 32], F32, tag="bptmp")
        nc.vector.tensor_mul(tmp[:, :nq], pbc[:, 32 : 32 + nq], sc_t[:, :nq])
        bi_t = cp.tile([128, 32], F32, name=f"bias_pre{j}")
        nc.vector.tensor_scalar(out=bi_t[:, :nq], in0=tmp[:, :nq],
                                scalar1=sb["gn_b"][:, gcol : gcol + 1],
                                scalar2=None, op0=OP.add)
        scale_pre[j] = sc_t
        bias_pre[j] = bi_t

    # ================= helpers =================
    def conv_taps(psum_flat, srcs_w, taps):
        """Accumulate conv into psum [128, 1024] viewed [128, 32, 32].
        srcs_w: list of (src_tile_flat, weight_ap_fn(tap_idx) -> [128,128]).
        taps: list of (dh, dw), center (0,0) must be included."""
        p3 = psum_flat.rearrange("p (h w) -> p h w", w=HW)
        order = sorted(range(len(taps)), key=lambda i: taps[i] != (0, 0))
        for bank in range(2):
            ops = []
            for oi in order:
                dh, dw = taps[oi]
                h0, h1 = max(0, -dh), HW - max(0, dh)
                w0, w1 = max(0, -dw), HW - max(0, dw)
                bh0, bh1 = max(h0, bank * 16), min(h1, bank * 16 + 16)
                if bh0 >= bh1:
                    continue
                for src, wfn in srcs_w:
                    s3 = src.rearrange("p (h w) -> p h w", w=HW)
                    ops.append((p3[:, bh0:bh1, w0:w1], wfn(oi),
                                s3[:, bh0 + dh : bh1 + dh, w0 + dw : w1 + dw]))
            for i, (o, w, r) in enumerate(ops):
                MM(o, w, r, start=(i == 0), stop=(i == len(ops) - 1),
                   skip_group_check=True)

    def gn_stats(psum_flat, gcol):
        """GroupNorm scale/bias [128,1] from conv output in PSUM."""
        st6 = tp.tile([128, 2, 6], F32, tag="st6")
        nc.vector.bn_stats(out=st6[:, 0, :], in_=psum_flat[:, 0:512])
        nc.vector.bn_stats(out=st6[:, 1, :], in_=psum_flat[:, 512:1024])
        mv = tp.tile([128, 2], F32, tag="mv")
        nc.vector.bn_aggr(out=mv, in_=st6)
        # mv = [mean_p, var_p] -> [mean_p, E[x^2]_p]
        m2 = tp.tile([128, 1], F32, tag="m2")
        nc.scalar.square(out=m2, in_=mv[:, 0:1])
        nc.vector.tensor_add(mv[:, 1:2], mv[:, 1:2], m2)
        # group-mean + broadcast to every partition in the group via G128
        pst = pp.tile([128, 64], F32, tag="small", bufs=2)
        MM(pst[:, 0:2], sb["g128s"], mv, start=True, stop=True)
        # pst = [group mean, group E[x^2]] broadcast per partition
        gm2 = tp.tile([128, 1], F32, tag="gm2")
        nc.scalar.square(out=gm2, in_=pst[:, 0:1])
        var = tp.tile([128, 1], F32, tag="var")
        nc.vector.tensor_sub(var, pst[:, 1:2], gm2)
        sd = tp.tile([128, 1], F32, tag="sd")
        nc.scalar.activation(out=sd, in_=var, func=AF.Sqrt, bias=sb["eps128"],
                             scale=1.0)
        rstd = tp.tile([128, 1], F32, tag="rstd")
        nc.vector.reciprocal(out=rstd, in_=sd)
        scale = tp.tile([128, 1], F32, tag="scale")
        nc.vector.tensor_mul(scale, sb["gn_g"][:, gcol : gcol + 1], rstd)
        mt = tp.tile([128, 1], F32, tag="mt")
        nc.vector.tensor_mul(mt, pst[:, 0:1], scale)
        biasv = tp.tile([128, 1], F32, tag="biasv")
        nc.vector.tensor_sub(biasv, sb["gn_b"][:, gcol : gcol + 1], mt)
        return scale, biasv

    TAPS_1x3 = [(0, -1), (0, 0), (0, 1)]
    TAPS_3x1 = [(-1, 0), (0, 0), (1, 0)]
    TAPS_3x3d3 = [(dh, dw) for dh in (-3, 0, 3) for dw in (-3, 0, 3)]
    TAPS_3x3 = [(dh, dw) for dh in (-1, 0, 1) for dw in (-1, 0, 1)]

    # ================= main loop: layer-major over query blocks =================
    QB = 5
    for qb0 in range(0, nq, QB):
        qs = list(range(qb0, min(qb0 + QB, nq)))
        x0s, trs = {}, {}
        cur = {}
        for q in qs:
            # ws row -> broadcast [128, 1024] via K=1 matmul
            wrow = qp.tile([1, S], F32, tag="wsrow", bufs=4)
            nc.sync.dma_start(out=wrow, in_=ws_sb[q : q + 1, :])
            pwsb = pp.tile([128, S], F32, tag="big", bufs=3)
            for ncol in range(2):
                MM(pwsb[:, ncol * 512 : (ncol + 1) * 512],
                   sb["ones"][0:1, :],
                   wrow[0:1, ncol * 512 : (ncol + 1) * 512],
                   start=True, stop=True)
            wsb = qp.tile([128, S], F32, tag="wsb_sb", bufs=3)
            nc.scalar.copy(out=wsb, in_=pwsb)

            t0 = qp.tile([128, S], F32, tag="t0", bufs=3)
            nc.vector.tensor_mul(t0, y_sb["0"], wsb)
            x0 = qp.tile([128, S], F16, tag="x0", bufs=6)
            nc.scalar.activation(out=x0, in_=t0, func=AF.Identity,
                                 bias=bias_pre["0"][:, q : q + 1],
                                 scale=scale_pre["0"][:, q : q + 1])
            t1 = qp.tile([128, S], F32, tag="t1", bufs=3)
            nc.gpsimd.tensor_mul(t1, y_sb["1"], wsb)
            x1a = qp.tile([128, S], F16, tag="xA", bufs=6)
            nc.scalar.activation(out=x1a, in_=t1, func=AF.Identity,
                                 bias=bias_pre["1"][:, q : q + 1],
                                 scale=scale_pre["1"][:, q : q + 1])
            tr = qp.tile([128, S], F32, tag="tr", bufs=6)
            nc.vector.tensor_mul(tr, y_sb["r"], wsb)
            x0s[q], trs[q], cur[q] = x0, tr, x1a

        # mid conv layers
        for wname, taps, gcol, otag in (("w1c1T", TAPS_1x3, 2, "xB"),
                                        ("w1c2T", TAPS_3x1, 3, "xA"),
                                        ("w1c3T", TAPS_3x3d3, 4, "xB")):
            nxt = {}
            for q in qs:
                ps = pp.tile([128, S], F32, tag="big", bufs=3)
                conv_taps(ps, [(cur[q], lambda t, w=wname: sb[w][:, 0, t, :])],
                          taps)
                s_, b_ = gn_stats(ps, gcol)
                xn = qp.tile([128, S], F16, tag=otag, bufs=6)
                nc.scalar.activation(out=xn, in_=ps, func=AF.Identity,
                                     bias=b_, scale=s_)
                nxt[q] = xn
            cur = nxt

        # cat conv + final fuse
        for q in qs:
            ps4 = pp.tile([128, S], F32, tag="big", bufs=3)
            conv_taps(ps4, [(x0s[q], lambda t: sb["wcatT"][:, 0, t, :]),
                            (cur[q], lambda t: sb["wcatT"][:, 1, t, :])],
                      TAPS_3x3)
            s4, b4 = gn_stats(ps4, 5)
            brt = tp.tile([128, 1], F32, tag="brt")
            nc.vector.tensor_scalar(out=brt, in0=b4,
                                    scalar1=bias_pre["r"][:, q : q + 1],
                                    scalar2=None, op0=OP.add)
            xr = qp.tile([128, S], F32, tag="xr", bufs=3)
            nc.scalar.activation(out=xr, in_=trs[q], func=AF.Identity,
                                 bias=brt, scale=scale_pre["r"][:, q : q + 1])
            u = qp.tile([128, S], F32, tag="u", bufs=3)
            nc.vector.scalar_tensor_tensor(out=u, in0=ps4, scalar=s4, in1=xr,
                                           op0=OP.mult, op1=OP.add)
            nc.scalar.activation(out=u, in_=u, func=AF.Relu)
            nc.sync.dma_start(out=out_h[:][q, :, :], in_=u)

    ctx.close()


# ----------------------------------------------------------------------------
# public entry point
# ----------------------------------------------------------------------------

_NC_CACHE = {}


def _get_program(nq=NQ):
    if nq not in _NC_CACHE:
        _NC_CACHE[nq] = build_program(nq)
    return _NC_CACHE[nq]


def make_in_maps(x, q, k, mask, params, nq=NQ):
    consts = _prep_consts(params)
    consts["gq"] = np.ascontiguousarray(consts["gq"][:, :nq])
    x = np.asarray(x, np.float32)
    q = np.asarray(q, np.float32)
    k = np.asarray(k, np.float32)
    mask = np.asarray(mask)
    in_maps = []
    for core in range(NCORES):
        b = core // (NCORES // B)
        q0 = (core % (NCORES // B)) * NQ  # stride 25 regardless of debug nq
        m = dict(consts)
        m["xk"] = np.ascontiguousarray(x[b].reshape(INC, S))
        m["kk"] = np.ascontiguousarray(k[b].reshape(QD, S))
        m["qT"] = np.ascontiguousarray(q[b, q0 : q0 + nq].T)
        m["mrow"] = np.ascontiguousarray(
            np.where(mask[b].reshape(1, S), np.float32(0.0), np.float32(1.0))
        ).astype(np.float32)
        in_maps.append(m)
    return in_maps


def kernel(x, q, k, mask, params):
    nc = _get_program(NQ)
    in_maps = make_in_maps(x, q, k, mask, params, NQ)
    res = run_bass_kernel_spmd(nc, in_maps, core_ids=list(range(NCORES)))
    outs = [res.results[i]["out"] for i in range(NCORES)]
    full = np.concatenate(outs, axis=0)  # [200, 128, 1024]
    return full.reshape(B * Q, OC, HW, HW)


if __name__ == "__main__":
    import reference

    inputs = reference.setup_inputs()
    out = kernel(**{k_: np.asarray(v) if not isinstance(v, dict) else v
                    for k_, v in inputs.items()})
    print("kernel out:", out.shape, out.dtype)
